# revision 9
# baseline (speedup 1.0000x reference)
"""EdgeBlock GNN message-passing kernel for 8 Trainium2 NeuronCores.

Sharding: edges split into 8 shards of 50000; triplets assigned to the core
owning index_ji (edge-aligned partitioning for the scatter-sum). Inside a
core, edges split into 3 "thirds" so every dma_gather / dma_scatter_add
index fits int16.

v2 (vs baseline): everything is plain fp16 (tolerance is 2e-2; fp16 keeps
us ~1e-3) which cuts matmuls 3x; LN sqrt is batched per block so the
scalar engine stays inside the sigmoid/tanh activation table (no
ACT_TABLE_LOAD thrash); the ji-equality selection matrix is built from an
iota + run-start/end compare on the vector engine instead of a PE
transpose round trip.

Per core, per third q:
  pre-phase : e_kj rows gathered in kj-bucket order (13 int16-local
              buckets) and redistributed to triplet order via
              dma_scatter_add with unique destination slots (exact).
  phase A   : transposed fp16 gathers of node_i/j/k and e_ji; e_kj rows
              read sequentially + PE-transposed; matmuls -> z [128,256]
              f32 PSUM; batched LN; sigmoid*tanh -> msg; run-block
              selection matmul combines equal-ji runs; combined rows
              written to msg_scratch (fp16).
  phase B   : per-edge gather of first-of-run msg rows (= segment sums),
              LN; c2 branch (node_i*node_j, matmul, LN, sig*tanh, LN);
              out = tanh(edge + c2_emb + c3_emb)  (fp16 out).
"""
import sys
for _p in ("/opt/trn_rl_repo", "/root/.axon_site/_ro/trn_rl_repo"):
    if _p not in sys.path:
        sys.path.insert(0, _p)

import numpy as np

P = 128
N_NODES, N_EDGES, N_TRIP = 20000, 400000, 500000
DN = DE = 128
N_CORES = 8
E_SH = N_EDGES // N_CORES            # 50000
Q = 3
E_THIRD = [16667, 16667, 16666]
E_OFF = [0, 16667, 33334]
ECH = 131                            # edge chunks per third
E_CAP = ECH * P                      # 16768
T_CH = 168                           # triplet chunks per third
T_CAP = T_CH * P                     # 21504
B_CH = 21                            # chunks per phase-A block
B_T = B_CH * P                       # 2688
NBLK = T_CH // B_CH                  # 8
KJ_NB = 13                           # kj buckets
KJ_W = 30770                         # bucket width (<= 32768)
KJ_QCH = 14                          # chunks per bucket quota
KJ_Q = KJ_QCH * P                    # 1792
KJ_CAP = KJ_NB * KJ_Q                # 23296
DUMP = 4096
EKJ_ROWS = T_CAP + DUMP              # 25600
MSG_ROWS = T_CAP + P                 # 21632
ZERO_ROW = T_CAP                     # 21504
EPS = 1e-5
EB_CH = [21] * 6 + [5]               # phase-B block chunk counts (=131)
# last third tapers off in smaller blocks so the final block's 3-pass
# compute latency (the post-last-gather tail) is short
EB_CH_LAST = [21] * 5 + [13, 8, 5]   # also 131


def _wrap16(vals, cap):
    """[cap] int array -> [128, cap/16] wrapped int16 (replicated 8x)."""
    assert cap % 16 == 0 and vals.shape[0] == cap
    assert vals.min() >= 0 and vals.max() <= 32767, (vals.min(), vals.max())
    w = np.zeros((16, cap // 16), np.int16)
    w[np.arange(cap) % 16, np.arange(cap) // 16] = vals.astype(np.int16)
    return np.tile(w, (8, 1))


def _pack_runs(ji_loc):
    """Positions for sorted ji_loc so no equal-value run crosses a 128
    boundary. Returns (pos array, total_padded_len)."""
    n = ji_loc.shape[0]
    starts = np.flatnonzero(np.r_[True, ji_loc[1:] != ji_loc[:-1]])
    lens = np.diff(np.r_[starts, n])
    assert lens.max() <= P, f"run length {lens.max()} > 128"
    pos = np.empty(n, np.int64)
    cur = 0
    for s, l in zip(starts, lens):
        if (cur % P) + l > P:
            cur = (cur // P + 1) * P
        pos[s:s + l] = cur + np.arange(l)
        cur += l
    return pos, ((cur + P - 1) // P) * P


def _prep_core(m, i, j, idx_i, idx_j, idx_k, ji, kj, trips_sorted, ji_sorted):
    lo = np.searchsorted(ji_sorted, m * E_SH, "left")
    hi = np.searchsorted(ji_sorted, (m + 1) * E_SH, "left")
    trips_m = trips_sorted[lo:hi]
    ji_m = ji_sorted[lo:hi]

    gi = np.zeros((Q, P, T_CAP // 16), np.int16)
    gj = np.zeros((Q, P, T_CAP // 16), np.int16)
    gk = np.zeros((Q, P, T_CAP // 16), np.int16)
    gji = np.zeros((Q, P, T_CAP // 16), np.int16)
    kidx = np.zeros((Q, P, KJ_CAP // 16), np.int16)
    sidx = np.zeros((Q, P, KJ_CAP // 16), np.int16)
    jsrt = np.zeros((Q, T_CH, P), np.float16)   # chunk-local run starts
    jend = np.zeros((Q, T_CH, P), np.float16)   # chunk-local run ends
    gmsg = np.zeros((Q, P, E_CAP // 16), np.int16)
    ci = np.zeros((Q, P, E_CAP // 16), np.int16)
    cj = np.zeros((Q, P, E_CAP // 16), np.int16)

    for q in range(Q):
        base = m * E_SH + E_OFF[q]
        qlo = np.searchsorted(ji_m, base, "left")
        qhi = np.searchsorted(ji_m, base + E_THIRD[q], "left")
        t = trips_m[qlo:qhi]                     # triplet ids, ji ascending
        jil = ji_m[qlo:qhi] - base               # [0, E_THIRD[q])
        pos, used = _pack_runs(jil)
        assert used <= T_CAP, f"third overflow {used} > {T_CAP}"

        ai = np.zeros(T_CAP, np.int64)
        aj = np.zeros(T_CAP, np.int64)
        ak = np.zeros(T_CAP, np.int64)
        aji = np.zeros(T_CAP, np.int64)
        ai[pos] = idx_i[t]
        aj[pos] = idx_j[t]
        ak[pos] = idx_k[t]
        aji[pos] = jil
        gi[q] = _wrap16(ai, T_CAP)
        gj[q] = _wrap16(aj, T_CAP)
        gk[q] = _wrap16(ak, T_CAP)
        gji[q] = _wrap16(aji, T_CAP)

        # chunk-local [start, end) of each slot's run; pads are singletons
        loc = np.arange(T_CAP) % P
        srt = loc.copy()
        end = loc + 1.0
        rstart = np.flatnonzero(np.r_[True, jil[1:] != jil[:-1]])
        rlen = np.diff(np.r_[rstart, jil.size])
        for s, l in zip(rstart, rlen):
            p0 = pos[s]
            srt[p0:p0 + l] = p0 % P
            end[p0:p0 + l] = p0 % P + l
        jsrt[q] = srt.reshape(T_CH, P).astype(np.float16)
        jend[q] = end.reshape(T_CH, P).astype(np.float16)

        fpos = np.full(E_CAP, ZERO_ROW, np.int64)
        fpos[jil[rstart]] = pos[rstart]
        gmsg[q] = _wrap16(fpos, E_CAP)

        e = np.arange(E_THIRD[q])
        bi = np.zeros(E_CAP, np.int64)
        bj = np.zeros(E_CAP, np.int64)
        bi[e] = i[base + e]
        bj[e] = j[base + e]
        ci[q] = _wrap16(bi, E_CAP)
        cj[q] = _wrap16(bj, E_CAP)

        kjq = kj[t]
        bkt = np.minimum(kjq // KJ_W, KJ_NB - 1)
        kv = np.zeros(KJ_CAP, np.int64)
        sv = np.full(KJ_CAP, -1, np.int64)
        pad_ctr = 0
        for b in range(KJ_NB):
            selb = np.flatnonzero(bkt == b)
            cnt = selb.size
            assert cnt <= KJ_Q, f"kj bucket overflow {cnt} > {KJ_Q}"
            o = b * KJ_Q
            kv[o:o + cnt] = kjq[selb] - b * KJ_W
            sv[o:o + cnt] = pos[selb]
            npad = KJ_Q - cnt
            sv[o + cnt:o + KJ_Q] = ZERO_ROW + pad_ctr + np.arange(npad)
            pad_ctr += npad
        assert pad_ctr <= DUMP, f"dump overflow {pad_ctr}"
        kidx[q] = _wrap16(kv, KJ_CAP)
        sidx[q] = _wrap16(sv, KJ_CAP)

    return dict(gidx_i=gi, gidx_j=gj, gidx_k=gk, gidx_ji=gji,
                kidx=kidx, sidx=sidx, jsrt=jsrt, jend=jend, gmsg=gmsg,
                cidx_i=ci, cidx_j=cj)


_CACHE = {}


def _build_kernel(fast):
    import concourse.bass as bass
    import concourse.bacc as bacc
    import concourse.tile as tile
    from concourse import mybir
    from concourse.masks import make_identity

    f32 = mybir.dt.float32
    fp16 = mybir.dt.float16
    i16 = mybir.dt.int16
    AF = mybir.ActivationFunctionType
    OP = mybir.AluOpType

    nc = bacc.Bacc("TRN2")
    node_h = nc.dram_tensor("node_h", [N_NODES, DN], fp16, kind="ExternalInput")
    edge_h = nc.dram_tensor("edge_h", [N_EDGES, DE], fp16, kind="ExternalInput")
    edge_sh = nc.dram_tensor("edge_sh", [E_SH, DE], fp16, kind="ExternalInput")
    edge_f = nc.dram_tensor("edge_f", [Q * E_CAP, DE], fp16, kind="ExternalInput")
    w3 = nc.dram_tensor("w3", [641, 256], fp16, kind="ExternalInput")
    w2 = nc.dram_tensor("w2", [129, 256], fp16, kind="ExternalInput")
    gbe3 = nc.dram_tensor("gbe3", [4, 256], f32, kind="ExternalInput")
    gbe2 = nc.dram_tensor("gbe2", [4, 128], f32, kind="ExternalInput")
    gidx_i = nc.dram_tensor("gidx_i", [Q, P, T_CAP // 16], i16, kind="ExternalInput")
    gidx_j = nc.dram_tensor("gidx_j", [Q, P, T_CAP // 16], i16, kind="ExternalInput")
    gidx_k = nc.dram_tensor("gidx_k", [Q, P, T_CAP // 16], i16, kind="ExternalInput")
    gidx_ji = nc.dram_tensor("gidx_ji", [Q, P, T_CAP // 16], i16, kind="ExternalInput")
    kidx = nc.dram_tensor("kidx", [Q, P, KJ_CAP // 16], i16, kind="ExternalInput")
    sidx = nc.dram_tensor("sidx", [Q, P, KJ_CAP // 16], i16, kind="ExternalInput")
    jsrt = nc.dram_tensor("jsrt", [Q, T_CH, P], fp16, kind="ExternalInput")
    jend = nc.dram_tensor("jend", [Q, T_CH, P], fp16, kind="ExternalInput")
    gmsg = nc.dram_tensor("gmsg", [Q, P, E_CAP // 16], i16, kind="ExternalInput")
    cidx_i = nc.dram_tensor("cidx_i", [Q, P, E_CAP // 16], i16, kind="ExternalInput")
    cidx_j = nc.dram_tensor("cidx_j", [Q, P, E_CAP // 16], i16, kind="ExternalInput")
    iot = nc.dram_tensor("iot", [P, P], fp16, kind="ExternalInput")
    out = nc.dram_tensor("out", [Q * E_CAP, DE], fp16, kind="ExternalOutput")
    # ExternalOutput: PJRT donates a zero-filled buffer, so the scatter
    # destination is pre-zeroed without on-device memset traffic.
    ekj_s = nc.dram_tensor("ekj_s", [Q, EKJ_ROWS, DE], fp16,
                           kind="ExternalOutput")
    msg_s = nc.dram_tensor("msg_s", [Q, MSG_ROWS, DE], fp16)

    with tile.TileContext(nc) as tc:
        with tc.tile_pool(name="const", bufs=1) as cp:
            ident = cp.tile([P, P], fp16)
            make_identity(nc, ident[:])
            ones1 = cp.tile([1, P], fp16)
            nc.vector.memset(ones1[:], 1.0)
            epst = cp.tile([P, 1], f32)
            nc.vector.memset(epst[:], EPS)
            iotaf = cp.tile([P, P], fp16)   # iotaf[a,b] = b
            nc.sync.dma_start(out=iotaf[:], in_=iot[:, :])
            w3t = {}
            for kc in range(5):
                wt = cp.tile([P, 256], fp16, name=f"w3_{kc}")
                nc.sync.dma_start(out=wt[:], in_=w3[kc * P:(kc + 1) * P, :])
                w3t[kc] = wt
            w3b = cp.tile([1, 256], fp16, name="w3b")
            nc.sync.dma_start(out=w3b[:], in_=w3[640:641, :])
            w2t = cp.tile([P, 256], fp16, name="w2t")
            nc.sync.dma_start(out=w2t[:], in_=w2[0:128, :])
            w2b = cp.tile([1, 256], fp16, name="w2b")
            nc.sync.dma_start(out=w2b[:], in_=w2[128:129, :])
            if not fast:
                g3 = cp.tile([P, 256], f32)
                be3 = cp.tile([P, 256], f32)
                g2 = cp.tile([P, 256], f32)
                be2 = cp.tile([P, 256], f32)
                g32 = cp.tile([P, 128], f32)
                be32 = cp.tile([P, 128], f32)
                g22 = cp.tile([P, 128], f32)
                be22 = cp.tile([P, 128], f32)
                for tl, src in ((g2, gbe3[0]), (be2, gbe3[1]),
                                (g3, gbe3[2]), (be3, gbe3[3]),
                                (g22, gbe2[0]), (be22, gbe2[1]),
                                (g32, gbe2[2]), (be32, gbe2[3])):
                    nc.gpsimd.dma_start(out=tl[:], in_=bass.AP(
                        tensor=src.tensor, offset=src.offset,
                        ap=[[0, P], src.ap[0]]))
            zmsg = cp.tile([P, P], fp16)
            nc.vector.memset(zmsg[:], 0.0)
            for q in range(Q):
                nc.sync.dma_start(
                    out=msg_s[q, ZERO_ROW:ZERO_ROW + P, :], in_=zmsg[:])

            # ---- kj pre-phase ----
            with tc.tile_pool(name="kjp", bufs=4) as kjp:
                for q in range(Q):
                    for b in range(KJ_NB):
                        cols = KJ_Q // 16
                        kt = kjp.tile([P, cols], i16, tag="kt")
                        nc.sync.dma_start(
                            out=kt[:], in_=kidx[q, :, b * cols:(b + 1) * cols])
                        st = kjp.tile([P, cols], i16, tag="st")
                        nc.sync.dma_start(
                            out=st[:], in_=sidx[q, :, b * cols:(b + 1) * cols])
                        kr = kjp.tile([P, KJ_QCH, P], fp16, tag="kr")
                        base = b * KJ_W
                        rows = min(KJ_W, N_EDGES - base)
                        nc.gpsimd.dma_gather(
                            out_ap=kr[:], in_ap=edge_h[base:base + rows, :],
                            idxs_ap=kt[:], num_idxs=KJ_Q, num_idxs_reg=KJ_Q,
                            elem_size=DE, transpose=False, single_packet=False)
                        nc.gpsimd.dma_scatter_add(
                            out_ap=ekj_s[q, :, :], in_ap=kr[:], idxs_ap=st[:],
                            num_idxs=KJ_Q, num_idxs_reg=KJ_Q, elem_size=DE,
                            single_packet=False)

            # ---- phase A ----
            with tc.tile_pool(name="abig", bufs=3) as abig, \
                 tc.tile_pool(name="aidx", bufs=3) as aidx, \
                 tc.tile_pool(name="asm", bufs=8) as asm, \
                 tc.tile_pool(name="aps", bufs=2, space="PSUM") as aps, \
                 tc.tile_pool(name="aps2", bufs=3, space="PSUM") as aps2:
                for q in range(Q):
                    for b in range(NBLK):
                        cc = B_T // 16
                        xts = []
                        for nm, src in (("i", gidx_i), ("j", gidx_j),
                                        ("k", gidx_k), ("ji", gidx_ji)):
                            it = aidx.tile([P, cc], i16, tag=f"ix{nm}")
                            nc.sync.dma_start(
                                out=it[:], in_=src[q, :, b * cc:(b + 1) * cc])
                            xt = abig.tile([P, 1, B_T], fp16, tag=f"xT{nm}")
                            if nm == "ji":
                                base = E_OFF[q]
                                inap = edge_sh[base:base + E_THIRD[q], :]
                            else:
                                inap = node_h[:, :]
                            nc.gpsimd.dma_gather(
                                out_ap=xt[:], in_ap=inap, idxs_ap=it[:],
                                num_idxs=B_T, num_idxs_reg=B_T,
                                elem_size=DN, transpose=True, single_packet=False)
                            xts.append(xt)
                        ek = abig.tile([P, B_CH, P], fp16, tag="ek")
                        nc.sync.dma_start(
                            out=ek[:], in_=ekj_s[q, b * B_T:(b + 1) * B_T, :]
                            .rearrange("(n p) f -> p n f", p=P))
                        jst = aidx.tile([P, B_CH], fp16, tag="jst")
                        nc.sync.dma_start(
                            out=jst[:], in_=jsrt[q, b * B_CH:(b + 1) * B_CH]
                            .rearrange("c p -> p c"))
                        jen = aidx.tile([P, B_CH], fp16, tag="jen")
                        nc.sync.dma_start(
                            out=jen[:], in_=jend[q, b * B_CH:(b + 1) * B_CH]
                            .rearrange("c p -> p c"))
                        msum = abig.tile([P, B_CH, P], fp16, tag="msum")
                        zb = abig.tile([P, B_CH, 256], fp16, tag="zb")
                        mvb = asm.tile([P, 2 * B_CH], f32, tag="mvb")
                        # pass 1: matmuls + stats + stash z
                        for c in range(B_CH):
                            cs = slice(c * P, (c + 1) * P)
                            tps = aps2.tile([P, P], fp16, tag="tps")
                            nc.tensor.transpose(
                                tps[:], ek[:, c, :], ident[:])
                            ekh = asm.tile([P, P], fp16, tag="ekT")
                            nc.vector.tensor_copy(ekh[:], tps[:])
                            z = aps.tile([P, 256], f32, tag="z")
                            for si in range(4):
                                nc.tensor.matmul(z[:], lhsT=xts[si][:, 0, cs],
                                                 rhs=w3t[si][:],
                                                 start=(si == 0), stop=False)
                            nc.tensor.matmul(z[:], lhsT=ekh[:], rhs=w3t[4][:],
                                             start=False, stop=False)
                            nc.tensor.matmul(z[:], lhsT=ones1[:], rhs=w3b[:],
                                             start=False, stop=True)
                            stats = asm.tile([P, 6], f32, tag="stats")
                            nc.vector.bn_stats(stats[:], z[:])
                            nc.vector.bn_aggr(mvb[:, 2 * c:2 * c + 2], stats[:])
                            nc.vector.tensor_copy(zb[:, c, :], z[:])
                        # batched LN scalars for the block
                        sdb = asm.tile([P, B_CH], f32, tag="sdb")
                        nc.scalar.activation(sdb[:], mvb[:, 1::2], AF.Sqrt,
                                             bias=epst[:], scale=1.0)
                        rsb = asm.tile([P, B_CH], f32, tag="rsb")
                        nc.vector.reciprocal(rsb[:], sdb[:])
                        nmb = asm.tile([P, B_CH], f32, tag="nmb")
                        nc.vector.scalar_tensor_tensor(
                            out=nmb[:], in0=mvb[:, 0::2], scalar=-1.0,
                            in1=rsb[:], op0=OP.mult, op1=OP.mult)
                        # pass 2: activations + run combine
                        for c in range(B_CH):
                            sg = asm.tile([P, P], fp16, tag="sg")
                            th = asm.tile([P, P], fp16, tag="th")
                            if fast:
                                nc.scalar.activation(sg[:], zb[:, c, 0:128],
                                                     AF.Sigmoid,
                                                     bias=nmb[:, c:c + 1],
                                                     scale=rsb[:, c:c + 1])
                                nc.scalar.activation(th[:], zb[:, c, 128:256],
                                                     AF.Tanh,
                                                     bias=nmb[:, c:c + 1],
                                                     scale=rsb[:, c:c + 1])
                            else:
                                nrm = asm.tile([P, 256], f32, tag="nrm")
                                nc.vector.tensor_scalar(
                                    out=nrm[:], in0=zb[:, c, :],
                                    scalar1=mvb[:, 2 * c:2 * c + 1],
                                    scalar2=rsb[:, c:c + 1],
                                    op0=OP.subtract, op1=OP.mult)
                                nc.vector.tensor_mul(nrm[:], nrm[:], g3[:])
                                nc.vector.tensor_add(nrm[:], nrm[:], be3[:])
                                nc.scalar.activation(sg[:], nrm[:, 0:128],
                                                     AF.Sigmoid)
                                nc.scalar.activation(th[:], nrm[:, 128:256],
                                                     AF.Tanh)
                            msg = asm.tile([P, P], fp16, tag="msg")
                            nc.vector.tensor_mul(msg[:], sg[:], th[:])
                            # sel[a,b] = (jsrt[a] <= b < jend[a])
                            s1 = asm.tile([P, P], fp16, tag="s1")
                            nc.vector.tensor_tensor(
                                out=s1[:], in0=iotaf[:],
                                in1=jst[:, c:c + 1].to_broadcast([P, P]),
                                op=OP.is_ge)
                            s2 = asm.tile([P, P], fp16, tag="s2")
                            nc.vector.tensor_tensor(
                                out=s2[:], in0=iotaf[:],
                                in1=jen[:, c:c + 1].to_broadcast([P, P]),
                                op=OP.is_lt)
                            sel = asm.tile([P, P], fp16, tag="sel")
                            nc.vector.tensor_mul(sel[:], s1[:], s2[:])
                            mm = aps2.tile([P, P], f32, tag="mm")
                            nc.tensor.matmul(mm[:], lhsT=sel[:],
                                             rhs=msg[:], start=True, stop=True)
                            nc.scalar.copy(msum[:, c, :], mm[:])
                        nc.sync.dma_start(
                            out=msg_s[q, b * B_T:(b + 1) * B_T, :].rearrange(
                                "(n p) f -> p n f", p=P), in_=msum[:])

            # ---- phase B ----
            with tc.tile_pool(name="bbig", bufs=3) as bbig, \
                 tc.tile_pool(name="bidx", bufs=3) as bidx, \
                 tc.tile_pool(name="bsm", bufs=8) as bsm, \
                 tc.tile_pool(name="bps", bufs=3, space="PSUM") as bps:
                for q in range(Q):
                    c0 = 0
                    for nch in (EB_CH_LAST if q == Q - 1 else EB_CH):
                        ne = nch * P
                        e0 = c0 * P
                        cc = ne // 16
                        mt = bidx.tile([P, cc], i16, tag="bmi")
                        nc.sync.dma_start(
                            out=mt[:], in_=gmsg[q, :, c0 * 8:c0 * 8 + cc])
                        msgt = bbig.tile([P, nch, P], fp16, tag="msgt")
                        nc.gpsimd.dma_gather(
                            out_ap=msgt[:], in_ap=msg_s[q, :, :], idxs_ap=mt[:],
                            num_idxs=ne, num_idxs_reg=ne, elem_size=DE,
                            transpose=False, single_packet=False)
                        nT = []
                        for nm, src in (("i", cidx_i), ("j", cidx_j)):
                            it = bidx.tile([P, cc], i16, tag=f"bix{nm}")
                            nc.sync.dma_start(
                                out=it[:], in_=src[q, :, c0 * 8:c0 * 8 + cc])
                            xt = bbig.tile([P, 1, ne], fp16, tag=f"bnT{nm}")
                            nc.gpsimd.dma_gather(
                                out_ap=xt[:], in_ap=node_h[:, :], idxs_ap=it[:],
                                num_idxs=ne, num_idxs_reg=ne,
                                elem_size=DN, transpose=True, single_packet=False)
                            nT.append(xt)
                        edt = bbig.tile([P, nch, P], fp16, tag="edt")
                        nc.sync.dma_start(
                            out=edt[:],
                            in_=edge_f[q * E_CAP + e0:q * E_CAP + e0 + ne, :]
                            .rearrange("(n p) f -> p n f", p=P))
                        outt = bbig.tile([P, nch, P], fp16, tag="outt")
                        z2b = bbig.tile([P, nch, 256], fp16, tag="z2b")
                        c2pb = bbig.tile([P, nch, P], fp16, tag="c2pb")
                        mv2b = bsm.tile([P, 2 * B_CH], f32, tag="mv2b")
                        mv3b = bsm.tile([P, 2 * B_CH], f32, tag="mv3b")
                        mv4b = bsm.tile([P, 2 * B_CH], f32, tag="mv4b")
                        # pass 1: c2 matmul + z2/c3 stats
                        for c in range(nch):
                            cs = slice(c * P, (c + 1) * P)
                            prod = bsm.tile([P, P], fp16, tag="prod")
                            nc.vector.tensor_mul(prod[:], nT[0][:, 0, cs],
                                                 nT[1][:, 0, cs])
                            z2 = bps.tile([P, 256], f32, tag="z2")
                            nc.tensor.matmul(z2[:], lhsT=prod[:], rhs=w2t[:],
                                             start=True, stop=False)
                            nc.tensor.matmul(z2[:], lhsT=ones1[:], rhs=w2b[:],
                                             start=False, stop=True)
                            st2 = bsm.tile([P, 6], f32, tag="st2")
                            nc.vector.bn_stats(st2[:], z2[:])
                            nc.vector.bn_aggr(mv2b[:, 2 * c:2 * c + 2], st2[:])
                            nc.vector.tensor_copy(z2b[:, c, :], z2[:])
                            st3 = bsm.tile([P, 6], f32, tag="st3")
                            nc.vector.bn_stats(st3[:], msgt[:, c, :])
                            nc.vector.bn_aggr(mv3b[:, 2 * c:2 * c + 2], st3[:])
                        sd2b = bsm.tile([P, B_CH], f32, tag="sd2b")
                        nc.scalar.activation(sd2b[:, :nch],
                                             mv2b[:, 1:2 * nch:2], AF.Sqrt,
                                             bias=epst[:], scale=1.0)
                        rs2b = bsm.tile([P, B_CH], f32, tag="rs2b")
                        nc.vector.reciprocal(rs2b[:, :nch], sd2b[:, :nch])
                        nm2b = bsm.tile([P, B_CH], f32, tag="nm2b")
                        nc.vector.scalar_tensor_tensor(
                            out=nm2b[:, :nch], in0=mv2b[:, 0:2 * nch:2],
                            scalar=-1.0,
                            in1=rs2b[:, :nch], op0=OP.mult, op1=OP.mult)
                        sd3b = bsm.tile([P, B_CH], f32, tag="sd3b")
                        nc.scalar.activation(sd3b[:, :nch],
                                             mv3b[:, 1:2 * nch:2], AF.Sqrt,
                                             bias=epst[:], scale=1.0)
                        rs3b = bsm.tile([P, B_CH], f32, tag="rs3b")
                        nc.vector.reciprocal(rs3b[:, :nch], sd3b[:, :nch])
                        nm3b = bsm.tile([P, B_CH], f32, tag="nm3b")
                        nc.vector.scalar_tensor_tensor(
                            out=nm3b[:, :nch], in0=mv3b[:, 0:2 * nch:2],
                            scalar=-1.0,
                            in1=rs3b[:, :nch], op0=OP.mult, op1=OP.mult)
                        # pass 2: c2 activations -> c2p + stats
                        for c in range(nch):
                            sg2 = bsm.tile([P, P], fp16, tag="sg2")
                            th2 = bsm.tile([P, P], fp16, tag="th2")
                            if fast:
                                nc.scalar.activation(sg2[:], z2b[:, c, 0:128],
                                                     AF.Sigmoid,
                                                     bias=nm2b[:, c:c + 1],
                                                     scale=rs2b[:, c:c + 1])
                                nc.scalar.activation(th2[:], z2b[:, c, 128:256],
                                                     AF.Tanh,
                                                     bias=nm2b[:, c:c + 1],
                                                     scale=rs2b[:, c:c + 1])
                            else:
                                nrm2 = bsm.tile([P, 256], f32, tag="nrm2")
                                nc.vector.tensor_scalar(
                                    out=nrm2[:], in0=z2b[:, c, :],
                                    scalar1=mv2b[:, 2 * c:2 * c + 1],
                                    scalar2=rs2b[:, c:c + 1],
                                    op0=OP.subtract, op1=OP.mult)
                                nc.vector.tensor_mul(nrm2[:], nrm2[:], g2[:])
                                nc.vector.tensor_add(nrm2[:], nrm2[:], be2[:])
                                nc.scalar.activation(sg2[:], nrm2[:, 0:128],
                                                     AF.Sigmoid)
                                nc.scalar.activation(th2[:], nrm2[:, 128:256],
                                                     AF.Tanh)
                            nc.vector.tensor_mul(c2pb[:, c, :], sg2[:], th2[:])
                            st4 = bsm.tile([P, 6], f32, tag="st4")
                            nc.vector.bn_stats(st4[:], c2pb[:, c, :])
                            nc.vector.bn_aggr(mv4b[:, 2 * c:2 * c + 2], st4[:])
                        sd4b = bsm.tile([P, B_CH], f32, tag="sd4b")
                        nc.scalar.activation(sd4b[:, :nch],
                                             mv4b[:, 1:2 * nch:2], AF.Sqrt,
                                             bias=epst[:], scale=1.0)
                        rs4b = bsm.tile([P, B_CH], f32, tag="rs4b")
                        nc.vector.reciprocal(rs4b[:, :nch], sd4b[:, :nch])
                        nm4b = bsm.tile([P, B_CH], f32, tag="nm4b")
                        nc.vector.scalar_tensor_tensor(
                            out=nm4b[:, :nch], in0=mv4b[:, 0:2 * nch:2],
                            scalar=-1.0,
                            in1=rs4b[:, :nch], op0=OP.mult, op1=OP.mult)
                        # pass 3: normalize + combine + tanh
                        for c in range(nch):
                            c2e = bsm.tile([P, P], f32, tag="c2e")
                            c3e = bsm.tile([P, P], f32, tag="c3e")
                            if fast:
                                nc.scalar.activation(c2e[:], c2pb[:, c, :],
                                                     AF.Identity,
                                                     bias=nm4b[:, c:c + 1],
                                                     scale=rs4b[:, c:c + 1])
                                nc.scalar.activation(c3e[:], msgt[:, c, :],
                                                     AF.Identity,
                                                     bias=nm3b[:, c:c + 1],
                                                     scale=rs3b[:, c:c + 1])
                            else:
                                nc.vector.tensor_scalar(
                                    out=c2e[:], in0=c2pb[:, c, :],
                                    scalar1=mv4b[:, 2 * c:2 * c + 1],
                                    scalar2=rs4b[:, c:c + 1],
                                    op0=OP.subtract, op1=OP.mult)
                                nc.vector.tensor_mul(c2e[:], c2e[:], g22[:])
                                nc.vector.tensor_add(c2e[:], c2e[:], be22[:])
                                nc.vector.tensor_scalar(
                                    out=c3e[:], in0=msgt[:, c, :],
                                    scalar1=mv3b[:, 2 * c:2 * c + 1],
                                    scalar2=rs3b[:, c:c + 1],
                                    op0=OP.subtract, op1=OP.mult)
                                nc.vector.tensor_mul(c3e[:], c3e[:], g32[:])
                                nc.vector.tensor_add(c3e[:], c3e[:], be32[:])
                            acc = bsm.tile([P, P], f32, tag="acc")
                            nc.vector.tensor_add(acc[:], c2e[:], c3e[:])
                            nc.vector.tensor_add(acc[:], acc[:], edt[:, c, :])
                            nc.scalar.activation(outt[:, c, :], acc[:], AF.Tanh)
                        nc.sync.dma_start(
                            out=out[q * E_CAP + e0:q * E_CAP + e0 + ne, :]
                            .rearrange("(n p) f -> p n f", p=P), in_=outt[:])
                        c0 += nch
    nc.finalize()
    return nc


def kernel(**inputs):
    from concourse.bass_utils import run_bass_kernel_spmd

    i = np.asarray(inputs["i"]).astype(np.int64)
    j = np.asarray(inputs["j"]).astype(np.int64)
    idx_i = np.asarray(inputs["index_i"]).astype(np.int64)
    idx_j = np.asarray(inputs["index_j"]).astype(np.int64)
    idx_k = np.asarray(inputs["index_k"]).astype(np.int64)
    ji = np.asarray(inputs["index_ji"]).astype(np.int64)
    kj = np.asarray(inputs["index_kj"]).astype(np.int64)
    node = np.asarray(inputs["node_embedding"], np.float32)
    edge = np.asarray(inputs["edge_embedding"], np.float32)

    node_h = node.astype(np.float16)
    edge_ha = edge.astype(np.float16)
    w3f = np.vstack([np.asarray(inputs["w_c3"], np.float32),
                     np.asarray(inputs["b_c3"], np.float32)[None]])
    w2f = np.vstack([np.asarray(inputs["w_c2"], np.float32),
                     np.asarray(inputs["b_c2"], np.float32)[None]])
    w3x = w3f.astype(np.float16)
    w2x = w2f.astype(np.float16)
    gbe3 = np.stack([np.asarray(inputs["g_bn_c2"], np.float32),
                     np.asarray(inputs["be_bn_c2"], np.float32),
                     np.asarray(inputs["g_bn_c3"], np.float32),
                     np.asarray(inputs["be_bn_c3"], np.float32)])
    gbe2 = np.stack([np.asarray(inputs["g_bn_c2_2"], np.float32),
                     np.asarray(inputs["be_bn_c2_2"], np.float32),
                     np.asarray(inputs["g_bn_c3_2"], np.float32),
                     np.asarray(inputs["be_bn_c3_2"], np.float32)])
    fast = (np.all(gbe3[0] == 1) and np.all(gbe3[2] == 1)
            and np.all(gbe2[0] == 1) and np.all(gbe2[2] == 1)
            and np.all(gbe3[1] == 0) and np.all(gbe3[3] == 0)
            and np.all(gbe2[1] == 0) and np.all(gbe2[3] == 0))

    order = np.argsort(ji, kind="stable")
    ji_sorted = ji[order]

    in_maps = []
    for m in range(N_CORES):
        d = _prep_core(m, i, j, idx_i, idx_j, idx_k, ji, kj, order, ji_sorted)
        egrid = np.zeros((Q * E_CAP, DE), np.float16)
        for q in range(Q):
            base = m * E_SH + E_OFF[q]
            egrid[q * E_CAP:q * E_CAP + E_THIRD[q]] = \
                edge_ha[base:base + E_THIRD[q]]
        d.update(iot=np.tile(np.arange(P, dtype=np.float16), (P, 1)),
                 node_h=node_h, edge_h=edge_ha,
                 edge_sh=edge_ha[m * E_SH:(m + 1) * E_SH].copy(),
                 edge_f=egrid, w3=w3x, w2=w2x, gbe3=gbe3, gbe2=gbe2)
        in_maps.append(d)

    key = ("k", fast)
    if key not in _CACHE:
        _CACHE[key] = _build_kernel(fast)
    nc = _CACHE[key]

    import os
    trace = bool(os.environ.get("KERNEL_TRACE"))
    res = run_bass_kernel_spmd(nc, in_maps, core_ids=list(range(N_CORES)),
                               trace=trace)
    global LAST_RESULT
    LAST_RESULT = res

    full = np.zeros((N_EDGES, DE), np.float32)
    for m in range(N_CORES):
        o = res.results[m]["out"].astype(np.float32)
        for q in range(Q):
            base = m * E_SH + E_OFF[q]
            full[base:base + E_THIRD[q]] = o[q * E_CAP:q * E_CAP + E_THIRD[q]]
    return full


# revision 10
# speedup vs baseline: 27.4835x; 27.4835x over previous
"""EdgeBlock GNN message-passing kernel for 8 Trainium2 NeuronCores.

Sharding: edges split into 8 shards of 50000; triplets assigned to the core
owning index_ji (edge-aligned partitioning for the scatter-sum). Inside a
core, edges split into 3 "thirds" so every dma_gather / dma_scatter_add
index fits int16.

v2 (vs baseline): everything is plain fp16 (tolerance is 2e-2; fp16 keeps
us ~1e-3) which cuts matmuls 3x; LN sqrt is batched per block so the
scalar engine stays inside the sigmoid/tanh activation table (no
ACT_TABLE_LOAD thrash); the ji-equality selection matrix is built from an
iota + run-start/end compare on the vector engine instead of a PE
transpose round trip.

Per core, per third q:
  pre-phase : e_kj rows gathered in kj-bucket order (13 int16-local
              buckets) and redistributed to triplet order via
              dma_scatter_add with unique destination slots (exact).
  phase A   : transposed fp16 gathers of node_i/j/k and e_ji; e_kj rows
              read sequentially + PE-transposed; matmuls -> z [128,256]
              f32 PSUM; batched LN; sigmoid*tanh -> msg; run-block
              selection matmul combines equal-ji runs; combined rows
              written to msg_scratch (fp16).
  phase B   : per-edge gather of first-of-run msg rows (= segment sums),
              LN; c2 branch (node_i*node_j, matmul, LN, sig*tanh, LN);
              out = tanh(edge + c2_emb + c3_emb)  (fp16 out).
"""
import sys
for _p in ("/opt/trn_rl_repo", "/root/.axon_site/_ro/trn_rl_repo"):
    if _p not in sys.path:
        sys.path.insert(0, _p)

import numpy as np

P = 128
N_NODES, N_EDGES, N_TRIP = 20000, 400000, 500000
DN = DE = 128
N_CORES = 8
E_SH = N_EDGES // N_CORES            # 50000
Q = 3
E_THIRD = [16667, 16667, 16666]
E_OFF = [0, 16667, 33334]
ECH = 131                            # edge chunks per third
E_CAP = ECH * P                      # 16768
T_CH = 168                           # triplet chunks per third
T_CAP = T_CH * P                     # 21504
B_CH = 21                            # chunks per phase-A block
B_T = B_CH * P                       # 2688
NBLK = T_CH // B_CH                  # 8
KJ_NB = 13                           # kj buckets
KJ_W = 30770                         # bucket width (<= 32768)
KJ_QCH = 14                          # chunks per bucket quota
KJ_Q = KJ_QCH * P                    # 1792
KJ_CAP = KJ_NB * KJ_Q                # 23296
DUMP = 4096
EKJ_ROWS = T_CAP + DUMP              # 25600
MSG_ROWS = T_CAP + P                 # 21632
ZERO_ROW = T_CAP                     # 21504
EPS = 1e-5
EB_CH = [21] * 6 + [5]               # phase-B block chunk counts (=131)


def _wrap16(vals, cap):
    """[cap] int array -> [128, cap/16] wrapped int16 (replicated 8x)."""
    assert cap % 16 == 0 and vals.shape[0] == cap
    assert vals.min() >= 0 and vals.max() <= 32767, (vals.min(), vals.max())
    w = np.zeros((16, cap // 16), np.int16)
    w[np.arange(cap) % 16, np.arange(cap) // 16] = vals.astype(np.int16)
    return np.tile(w, (8, 1))


def _pack_runs(ji_loc):
    """Positions for sorted ji_loc so no equal-value run crosses a 128
    boundary. Returns (pos array, total_padded_len)."""
    n = ji_loc.shape[0]
    starts = np.flatnonzero(np.r_[True, ji_loc[1:] != ji_loc[:-1]])
    lens = np.diff(np.r_[starts, n])
    assert lens.max() <= P, f"run length {lens.max()} > 128"
    pos = np.empty(n, np.int64)
    cur = 0
    for s, l in zip(starts, lens):
        if (cur % P) + l > P:
            cur = (cur // P + 1) * P
        pos[s:s + l] = cur + np.arange(l)
        cur += l
    return pos, ((cur + P - 1) // P) * P


def _prep_core(m, i, j, idx_i, idx_j, idx_k, ji, kj, trips_sorted, ji_sorted):
    lo = np.searchsorted(ji_sorted, m * E_SH, "left")
    hi = np.searchsorted(ji_sorted, (m + 1) * E_SH, "left")
    trips_m = trips_sorted[lo:hi]
    ji_m = ji_sorted[lo:hi]

    gi = np.zeros((Q, P, T_CAP // 16), np.int16)
    gj = np.zeros((Q, P, T_CAP // 16), np.int16)
    gk = np.zeros((Q, P, T_CAP // 16), np.int16)
    gji = np.zeros((Q, P, T_CAP // 16), np.int16)
    kidx = np.zeros((Q, P, KJ_CAP // 16), np.int16)
    sidx = np.zeros((Q, P, KJ_CAP // 16), np.int16)
    jsrt = np.zeros((Q, T_CH, P), np.float16)   # chunk-local run starts
    jend = np.zeros((Q, T_CH, P), np.float16)   # chunk-local run ends
    gmsg = np.zeros((Q, P, E_CAP // 16), np.int16)
    ci = np.zeros((Q, P, E_CAP // 16), np.int16)
    cj = np.zeros((Q, P, E_CAP // 16), np.int16)

    for q in range(Q):
        base = m * E_SH + E_OFF[q]
        qlo = np.searchsorted(ji_m, base, "left")
        qhi = np.searchsorted(ji_m, base + E_THIRD[q], "left")
        t = trips_m[qlo:qhi]                     # triplet ids, ji ascending
        jil = ji_m[qlo:qhi] - base               # [0, E_THIRD[q])
        pos, used = _pack_runs(jil)
        assert used <= T_CAP, f"third overflow {used} > {T_CAP}"

        ai = np.zeros(T_CAP, np.int64)
        aj = np.zeros(T_CAP, np.int64)
        ak = np.zeros(T_CAP, np.int64)
        aji = np.zeros(T_CAP, np.int64)
        ai[pos] = idx_i[t]
        aj[pos] = idx_j[t]
        ak[pos] = idx_k[t]
        aji[pos] = jil
        gi[q] = _wrap16(ai, T_CAP)
        gj[q] = _wrap16(aj, T_CAP)
        gk[q] = _wrap16(ak, T_CAP)
        gji[q] = _wrap16(aji, T_CAP)

        # chunk-local [start, end) of each slot's run; pads are singletons
        loc = np.arange(T_CAP) % P
        srt = loc.copy()
        end = loc + 1.0
        rstart = np.flatnonzero(np.r_[True, jil[1:] != jil[:-1]])
        rlen = np.diff(np.r_[rstart, jil.size])
        for s, l in zip(rstart, rlen):
            p0 = pos[s]
            srt[p0:p0 + l] = p0 % P
            end[p0:p0 + l] = p0 % P + l
        jsrt[q] = srt.reshape(T_CH, P).astype(np.float16)
        jend[q] = end.reshape(T_CH, P).astype(np.float16)

        fpos = np.full(E_CAP, ZERO_ROW, np.int64)
        fpos[jil[rstart]] = pos[rstart]
        gmsg[q] = _wrap16(fpos, E_CAP)

        e = np.arange(E_THIRD[q])
        bi = np.zeros(E_CAP, np.int64)
        bj = np.zeros(E_CAP, np.int64)
        bi[e] = i[base + e]
        bj[e] = j[base + e]
        ci[q] = _wrap16(bi, E_CAP)
        cj[q] = _wrap16(bj, E_CAP)

        kjq = kj[t]
        bkt = np.minimum(kjq // KJ_W, KJ_NB - 1)
        kv = np.zeros(KJ_CAP, np.int64)
        sv = np.full(KJ_CAP, -1, np.int64)
        pad_ctr = 0
        for b in range(KJ_NB):
            selb = np.flatnonzero(bkt == b)
            cnt = selb.size
            assert cnt <= KJ_Q, f"kj bucket overflow {cnt} > {KJ_Q}"
            o = b * KJ_Q
            kv[o:o + cnt] = kjq[selb] - b * KJ_W
            sv[o:o + cnt] = pos[selb]
            npad = KJ_Q - cnt
            sv[o + cnt:o + KJ_Q] = ZERO_ROW + pad_ctr + np.arange(npad)
            pad_ctr += npad
        assert pad_ctr <= DUMP, f"dump overflow {pad_ctr}"
        kidx[q] = _wrap16(kv, KJ_CAP)
        sidx[q] = _wrap16(sv, KJ_CAP)

    return dict(gidx_i=gi, gidx_j=gj, gidx_k=gk, gidx_ji=gji,
                kidx=kidx, sidx=sidx, jsrt=jsrt, jend=jend, gmsg=gmsg,
                cidx_i=ci, cidx_j=cj)


_CACHE = {}


def _build_kernel(fast):
    import concourse.bass as bass
    import concourse.bacc as bacc
    import concourse.tile as tile
    from concourse import mybir
    from concourse.masks import make_identity

    f32 = mybir.dt.float32
    fp16 = mybir.dt.float16
    i16 = mybir.dt.int16
    AF = mybir.ActivationFunctionType
    OP = mybir.AluOpType

    nc = bacc.Bacc("TRN2")
    node_h = nc.dram_tensor("node_h", [N_NODES, DN], fp16, kind="ExternalInput")
    edge_h = nc.dram_tensor("edge_h", [N_EDGES, DE], fp16, kind="ExternalInput")
    edge_sh = nc.dram_tensor("edge_sh", [E_SH, DE], fp16, kind="ExternalInput")
    edge_f = nc.dram_tensor("edge_f", [Q * E_CAP, DE], fp16, kind="ExternalInput")
    w3 = nc.dram_tensor("w3", [641, 256], fp16, kind="ExternalInput")
    w2 = nc.dram_tensor("w2", [129, 256], fp16, kind="ExternalInput")
    gbe3 = nc.dram_tensor("gbe3", [4, 256], f32, kind="ExternalInput")
    gbe2 = nc.dram_tensor("gbe2", [4, 128], f32, kind="ExternalInput")
    gidx_i = nc.dram_tensor("gidx_i", [Q, P, T_CAP // 16], i16, kind="ExternalInput")
    gidx_j = nc.dram_tensor("gidx_j", [Q, P, T_CAP // 16], i16, kind="ExternalInput")
    gidx_k = nc.dram_tensor("gidx_k", [Q, P, T_CAP // 16], i16, kind="ExternalInput")
    gidx_ji = nc.dram_tensor("gidx_ji", [Q, P, T_CAP // 16], i16, kind="ExternalInput")
    kidx = nc.dram_tensor("kidx", [Q, P, KJ_CAP // 16], i16, kind="ExternalInput")
    sidx = nc.dram_tensor("sidx", [Q, P, KJ_CAP // 16], i16, kind="ExternalInput")
    jsrt = nc.dram_tensor("jsrt", [Q, T_CH, P], fp16, kind="ExternalInput")
    jend = nc.dram_tensor("jend", [Q, T_CH, P], fp16, kind="ExternalInput")
    gmsg = nc.dram_tensor("gmsg", [Q, P, E_CAP // 16], i16, kind="ExternalInput")
    cidx_i = nc.dram_tensor("cidx_i", [Q, P, E_CAP // 16], i16, kind="ExternalInput")
    cidx_j = nc.dram_tensor("cidx_j", [Q, P, E_CAP // 16], i16, kind="ExternalInput")
    iot = nc.dram_tensor("iot", [P, P], fp16, kind="ExternalInput")
    out = nc.dram_tensor("out", [Q * E_CAP, DE], fp16, kind="ExternalOutput")
    # ExternalOutput: PJRT donates a zero-filled buffer, so the scatter
    # destination is pre-zeroed without on-device memset traffic.
    ekj_s = nc.dram_tensor("ekj_s", [Q, EKJ_ROWS, DE], fp16,
                           kind="ExternalOutput")
    msg_s = nc.dram_tensor("msg_s", [Q, MSG_ROWS, DE], fp16)

    with tile.TileContext(nc) as tc:
        with tc.tile_pool(name="const", bufs=1) as cp:
            ident = cp.tile([P, P], fp16)
            make_identity(nc, ident[:])
            ones1 = cp.tile([1, P], fp16)
            nc.vector.memset(ones1[:], 1.0)
            epst = cp.tile([P, 1], f32)
            nc.vector.memset(epst[:], EPS)
            iotaf = cp.tile([P, P], fp16)   # iotaf[a,b] = b
            nc.sync.dma_start(out=iotaf[:], in_=iot[:, :])
            w3t = {}
            for kc in range(5):
                wt = cp.tile([P, 256], fp16, name=f"w3_{kc}")
                nc.sync.dma_start(out=wt[:], in_=w3[kc * P:(kc + 1) * P, :])
                w3t[kc] = wt
            w3b = cp.tile([1, 256], fp16, name="w3b")
            nc.sync.dma_start(out=w3b[:], in_=w3[640:641, :])
            w2t = cp.tile([P, 256], fp16, name="w2t")
            nc.sync.dma_start(out=w2t[:], in_=w2[0:128, :])
            w2b = cp.tile([1, 256], fp16, name="w2b")
            nc.sync.dma_start(out=w2b[:], in_=w2[128:129, :])
            if not fast:
                g3 = cp.tile([P, 256], f32)
                be3 = cp.tile([P, 256], f32)
                g2 = cp.tile([P, 256], f32)
                be2 = cp.tile([P, 256], f32)
                g32 = cp.tile([P, 128], f32)
                be32 = cp.tile([P, 128], f32)
                g22 = cp.tile([P, 128], f32)
                be22 = cp.tile([P, 128], f32)
                for tl, src in ((g2, gbe3[0]), (be2, gbe3[1]),
                                (g3, gbe3[2]), (be3, gbe3[3]),
                                (g22, gbe2[0]), (be22, gbe2[1]),
                                (g32, gbe2[2]), (be32, gbe2[3])):
                    nc.gpsimd.dma_start(out=tl[:], in_=bass.AP(
                        tensor=src.tensor, offset=src.offset,
                        ap=[[0, P], src.ap[0]]))
            zmsg = cp.tile([P, P], fp16)
            nc.vector.memset(zmsg[:], 0.0)
            for q in range(Q):
                nc.sync.dma_start(
                    out=msg_s[q, ZERO_ROW:ZERO_ROW + P, :], in_=zmsg[:])

            # ---- kj pre-phase ----
            with tc.tile_pool(name="kjp", bufs=4) as kjp:
                for q in range(Q):
                    for b in range(KJ_NB):
                        cols = KJ_Q // 16
                        kt = kjp.tile([P, cols], i16, tag="kt")
                        nc.sync.dma_start(
                            out=kt[:], in_=kidx[q, :, b * cols:(b + 1) * cols])
                        st = kjp.tile([P, cols], i16, tag="st")
                        nc.sync.dma_start(
                            out=st[:], in_=sidx[q, :, b * cols:(b + 1) * cols])
                        kr = kjp.tile([P, KJ_QCH, P], fp16, tag="kr")
                        base = b * KJ_W
                        rows = min(KJ_W, N_EDGES - base)
                        nc.gpsimd.dma_gather(
                            out_ap=kr[:], in_ap=edge_h[base:base + rows, :],
                            idxs_ap=kt[:], num_idxs=KJ_Q, num_idxs_reg=KJ_Q,
                            elem_size=DE, transpose=False, single_packet=False)
                        nc.gpsimd.dma_scatter_add(
                            out_ap=ekj_s[q, :, :], in_ap=kr[:], idxs_ap=st[:],
                            num_idxs=KJ_Q, num_idxs_reg=KJ_Q, elem_size=DE,
                            single_packet=False)

            # ---- phase A ----
            with tc.tile_pool(name="abig", bufs=3) as abig, \
                 tc.tile_pool(name="aidx", bufs=3) as aidx, \
                 tc.tile_pool(name="asm", bufs=8) as asm, \
                 tc.tile_pool(name="aps", bufs=2, space="PSUM") as aps, \
                 tc.tile_pool(name="aps2", bufs=3, space="PSUM") as aps2:
                for q in range(Q):
                    for b in range(NBLK):
                        cc = B_T // 16
                        xts = []
                        for nm, src in (("i", gidx_i), ("j", gidx_j),
                                        ("k", gidx_k), ("ji", gidx_ji)):
                            it = aidx.tile([P, cc], i16, tag=f"ix{nm}")
                            nc.sync.dma_start(
                                out=it[:], in_=src[q, :, b * cc:(b + 1) * cc])
                            xt = abig.tile([P, 1, B_T], fp16, tag=f"xT{nm}")
                            if nm == "ji":
                                base = E_OFF[q]
                                inap = edge_sh[base:base + E_THIRD[q], :]
                            else:
                                inap = node_h[:, :]
                            nc.gpsimd.dma_gather(
                                out_ap=xt[:], in_ap=inap, idxs_ap=it[:],
                                num_idxs=B_T, num_idxs_reg=B_T,
                                elem_size=DN, transpose=True, single_packet=False)
                            xts.append(xt)
                        ek = abig.tile([P, B_CH, P], fp16, tag="ek")
                        nc.sync.dma_start(
                            out=ek[:], in_=ekj_s[q, b * B_T:(b + 1) * B_T, :]
                            .rearrange("(n p) f -> p n f", p=P))
                        jst = aidx.tile([P, B_CH], fp16, tag="jst")
                        nc.sync.dma_start(
                            out=jst[:], in_=jsrt[q, b * B_CH:(b + 1) * B_CH]
                            .rearrange("c p -> p c"))
                        jen = aidx.tile([P, B_CH], fp16, tag="jen")
                        nc.sync.dma_start(
                            out=jen[:], in_=jend[q, b * B_CH:(b + 1) * B_CH]
                            .rearrange("c p -> p c"))
                        msum = abig.tile([P, B_CH, P], fp16, tag="msum")
                        zb = abig.tile([P, B_CH, 256], fp16, tag="zb")
                        mvb = asm.tile([P, 2 * B_CH], f32, tag="mvb")
                        # pass 1: matmuls + stats + stash z
                        for c in range(B_CH):
                            cs = slice(c * P, (c + 1) * P)
                            tps = aps2.tile([P, P], fp16, tag="tps")
                            nc.tensor.transpose(
                                tps[:], ek[:, c, :], ident[:])
                            ekh = asm.tile([P, P], fp16, tag="ekT")
                            nc.vector.tensor_copy(ekh[:], tps[:])
                            z = aps.tile([P, 256], f32, tag="z")
                            for si in range(4):
                                nc.tensor.matmul(z[:], lhsT=xts[si][:, 0, cs],
                                                 rhs=w3t[si][:],
                                                 start=(si == 0), stop=False)
                            nc.tensor.matmul(z[:], lhsT=ekh[:], rhs=w3t[4][:],
                                             start=False, stop=False)
                            nc.tensor.matmul(z[:], lhsT=ones1[:], rhs=w3b[:],
                                             start=False, stop=True)
                            stats = asm.tile([P, 6], f32, tag="stats")
                            nc.vector.bn_stats(stats[:], z[:])
                            nc.vector.bn_aggr(mvb[:, 2 * c:2 * c + 2], stats[:])
                            nc.vector.tensor_copy(zb[:, c, :], z[:])
                        # batched LN scalars for the block
                        sdb = asm.tile([P, B_CH], f32, tag="sdb")
                        nc.scalar.activation(sdb[:], mvb[:, 1::2], AF.Sqrt,
                                             bias=epst[:], scale=1.0)
                        rsb = asm.tile([P, B_CH], f32, tag="rsb")
                        nc.vector.reciprocal(rsb[:], sdb[:])
                        nmb = asm.tile([P, B_CH], f32, tag="nmb")
                        nc.vector.scalar_tensor_tensor(
                            out=nmb[:], in0=mvb[:, 0::2], scalar=-1.0,
                            in1=rsb[:], op0=OP.mult, op1=OP.mult)
                        # pass 2: activations + run combine
                        for c in range(B_CH):
                            sg = asm.tile([P, P], fp16, tag="sg")
                            th = asm.tile([P, P], fp16, tag="th")
                            if fast:
                                nc.scalar.activation(sg[:], zb[:, c, 0:128],
                                                     AF.Sigmoid,
                                                     bias=nmb[:, c:c + 1],
                                                     scale=rsb[:, c:c + 1])
                                nc.scalar.activation(th[:], zb[:, c, 128:256],
                                                     AF.Tanh,
                                                     bias=nmb[:, c:c + 1],
                                                     scale=rsb[:, c:c + 1])
                            else:
                                nrm = asm.tile([P, 256], f32, tag="nrm")
                                nc.vector.tensor_scalar(
                                    out=nrm[:], in0=zb[:, c, :],
                                    scalar1=mvb[:, 2 * c:2 * c + 1],
                                    scalar2=rsb[:, c:c + 1],
                                    op0=OP.subtract, op1=OP.mult)
                                nc.vector.tensor_mul(nrm[:], nrm[:], g3[:])
                                nc.vector.tensor_add(nrm[:], nrm[:], be3[:])
                                nc.scalar.activation(sg[:], nrm[:, 0:128],
                                                     AF.Sigmoid)
                                nc.scalar.activation(th[:], nrm[:, 128:256],
                                                     AF.Tanh)
                            msg = asm.tile([P, P], fp16, tag="msg")
                            nc.vector.tensor_mul(msg[:], sg[:], th[:])
                            # sel[a,b] = (jsrt[a] <= b < jend[a])
                            s1 = asm.tile([P, P], fp16, tag="s1")
                            nc.vector.tensor_tensor(
                                out=s1[:], in0=iotaf[:],
                                in1=jst[:, c:c + 1].to_broadcast([P, P]),
                                op=OP.is_ge)
                            s2 = asm.tile([P, P], fp16, tag="s2")
                            nc.vector.tensor_tensor(
                                out=s2[:], in0=iotaf[:],
                                in1=jen[:, c:c + 1].to_broadcast([P, P]),
                                op=OP.is_lt)
                            sel = asm.tile([P, P], fp16, tag="sel")
                            nc.vector.tensor_mul(sel[:], s1[:], s2[:])
                            mm = aps2.tile([P, P], f32, tag="mm")
                            nc.tensor.matmul(mm[:], lhsT=sel[:],
                                             rhs=msg[:], start=True, stop=True)
                            nc.scalar.copy(msum[:, c, :], mm[:])
                        nc.sync.dma_start(
                            out=msg_s[q, b * B_T:(b + 1) * B_T, :].rearrange(
                                "(n p) f -> p n f", p=P), in_=msum[:])

            # ---- phase B ----
            with tc.tile_pool(name="bbig", bufs=3) as bbig, \
                 tc.tile_pool(name="bidx", bufs=3) as bidx, \
                 tc.tile_pool(name="bsm", bufs=8) as bsm, \
                 tc.tile_pool(name="bps", bufs=3, space="PSUM") as bps:
                for q in range(Q):
                    c0 = 0
                    for nch in EB_CH:
                        ne = nch * P
                        e0 = c0 * P
                        cc = ne // 16
                        mt = bidx.tile([P, cc], i16, tag="bmi")
                        nc.sync.dma_start(
                            out=mt[:], in_=gmsg[q, :, c0 * 8:c0 * 8 + cc])
                        msgt = bbig.tile([P, nch, P], fp16, tag="msgt")
                        nc.gpsimd.dma_gather(
                            out_ap=msgt[:], in_ap=msg_s[q, :, :], idxs_ap=mt[:],
                            num_idxs=ne, num_idxs_reg=ne, elem_size=DE,
                            transpose=False, single_packet=False)
                        nT = []
                        for nm, src in (("i", cidx_i), ("j", cidx_j)):
                            it = bidx.tile([P, cc], i16, tag=f"bix{nm}")
                            nc.sync.dma_start(
                                out=it[:], in_=src[q, :, c0 * 8:c0 * 8 + cc])
                            xt = bbig.tile([P, 1, ne], fp16, tag=f"bnT{nm}")
                            nc.gpsimd.dma_gather(
                                out_ap=xt[:], in_ap=node_h[:, :], idxs_ap=it[:],
                                num_idxs=ne, num_idxs_reg=ne,
                                elem_size=DN, transpose=True, single_packet=False)
                            nT.append(xt)
                        edt = bbig.tile([P, nch, P], fp16, tag="edt")
                        nc.sync.dma_start(
                            out=edt[:],
                            in_=edge_f[q * E_CAP + e0:q * E_CAP + e0 + ne, :]
                            .rearrange("(n p) f -> p n f", p=P))
                        outt = bbig.tile([P, nch, P], fp16, tag="outt")
                        z2b = bbig.tile([P, nch, 256], fp16, tag="z2b")
                        c2pb = bbig.tile([P, nch, P], fp16, tag="c2pb")
                        mv2b = bsm.tile([P, 2 * B_CH], f32, tag="mv2b")
                        mv3b = bsm.tile([P, 2 * B_CH], f32, tag="mv3b")
                        mv4b = bsm.tile([P, 2 * B_CH], f32, tag="mv4b")
                        # pass 1: c2 matmul + z2/c3 stats
                        for c in range(nch):
                            cs = slice(c * P, (c + 1) * P)
                            prod = bsm.tile([P, P], fp16, tag="prod")
                            nc.vector.tensor_mul(prod[:], nT[0][:, 0, cs],
                                                 nT[1][:, 0, cs])
                            z2 = bps.tile([P, 256], f32, tag="z2")
                            nc.tensor.matmul(z2[:], lhsT=prod[:], rhs=w2t[:],
                                             start=True, stop=False)
                            nc.tensor.matmul(z2[:], lhsT=ones1[:], rhs=w2b[:],
                                             start=False, stop=True)
                            st2 = bsm.tile([P, 6], f32, tag="st2")
                            nc.vector.bn_stats(st2[:], z2[:])
                            nc.vector.bn_aggr(mv2b[:, 2 * c:2 * c + 2], st2[:])
                            nc.vector.tensor_copy(z2b[:, c, :], z2[:])
                            st3 = bsm.tile([P, 6], f32, tag="st3")
                            nc.vector.bn_stats(st3[:], msgt[:, c, :])
                            nc.vector.bn_aggr(mv3b[:, 2 * c:2 * c + 2], st3[:])
                        sd2b = bsm.tile([P, B_CH], f32, tag="sd2b")
                        nc.scalar.activation(sd2b[:, :nch],
                                             mv2b[:, 1:2 * nch:2], AF.Sqrt,
                                             bias=epst[:], scale=1.0)
                        rs2b = bsm.tile([P, B_CH], f32, tag="rs2b")
                        nc.vector.reciprocal(rs2b[:, :nch], sd2b[:, :nch])
                        nm2b = bsm.tile([P, B_CH], f32, tag="nm2b")
                        nc.vector.scalar_tensor_tensor(
                            out=nm2b[:, :nch], in0=mv2b[:, 0:2 * nch:2],
                            scalar=-1.0,
                            in1=rs2b[:, :nch], op0=OP.mult, op1=OP.mult)
                        sd3b = bsm.tile([P, B_CH], f32, tag="sd3b")
                        nc.scalar.activation(sd3b[:, :nch],
                                             mv3b[:, 1:2 * nch:2], AF.Sqrt,
                                             bias=epst[:], scale=1.0)
                        rs3b = bsm.tile([P, B_CH], f32, tag="rs3b")
                        nc.vector.reciprocal(rs3b[:, :nch], sd3b[:, :nch])
                        nm3b = bsm.tile([P, B_CH], f32, tag="nm3b")
                        nc.vector.scalar_tensor_tensor(
                            out=nm3b[:, :nch], in0=mv3b[:, 0:2 * nch:2],
                            scalar=-1.0,
                            in1=rs3b[:, :nch], op0=OP.mult, op1=OP.mult)
                        # pass 2: c2 activations -> c2p + stats
                        for c in range(nch):
                            sg2 = bsm.tile([P, P], fp16, tag="sg2")
                            th2 = bsm.tile([P, P], fp16, tag="th2")
                            if fast:
                                nc.scalar.activation(sg2[:], z2b[:, c, 0:128],
                                                     AF.Sigmoid,
                                                     bias=nm2b[:, c:c + 1],
                                                     scale=rs2b[:, c:c + 1])
                                nc.scalar.activation(th2[:], z2b[:, c, 128:256],
                                                     AF.Tanh,
                                                     bias=nm2b[:, c:c + 1],
                                                     scale=rs2b[:, c:c + 1])
                            else:
                                nrm2 = bsm.tile([P, 256], f32, tag="nrm2")
                                nc.vector.tensor_scalar(
                                    out=nrm2[:], in0=z2b[:, c, :],
                                    scalar1=mv2b[:, 2 * c:2 * c + 1],
                                    scalar2=rs2b[:, c:c + 1],
                                    op0=OP.subtract, op1=OP.mult)
                                nc.vector.tensor_mul(nrm2[:], nrm2[:], g2[:])
                                nc.vector.tensor_add(nrm2[:], nrm2[:], be2[:])
                                nc.scalar.activation(sg2[:], nrm2[:, 0:128],
                                                     AF.Sigmoid)
                                nc.scalar.activation(th2[:], nrm2[:, 128:256],
                                                     AF.Tanh)
                            nc.vector.tensor_mul(c2pb[:, c, :], sg2[:], th2[:])
                            st4 = bsm.tile([P, 6], f32, tag="st4")
                            nc.vector.bn_stats(st4[:], c2pb[:, c, :])
                            nc.vector.bn_aggr(mv4b[:, 2 * c:2 * c + 2], st4[:])
                        sd4b = bsm.tile([P, B_CH], f32, tag="sd4b")
                        nc.scalar.activation(sd4b[:, :nch],
                                             mv4b[:, 1:2 * nch:2], AF.Sqrt,
                                             bias=epst[:], scale=1.0)
                        rs4b = bsm.tile([P, B_CH], f32, tag="rs4b")
                        nc.vector.reciprocal(rs4b[:, :nch], sd4b[:, :nch])
                        nm4b = bsm.tile([P, B_CH], f32, tag="nm4b")
                        nc.vector.scalar_tensor_tensor(
                            out=nm4b[:, :nch], in0=mv4b[:, 0:2 * nch:2],
                            scalar=-1.0,
                            in1=rs4b[:, :nch], op0=OP.mult, op1=OP.mult)
                        # pass 3: normalize + combine + tanh
                        for c in range(nch):
                            c2e = bsm.tile([P, P], f32, tag="c2e")
                            c3e = bsm.tile([P, P], f32, tag="c3e")
                            if fast:
                                nc.scalar.activation(c2e[:], c2pb[:, c, :],
                                                     AF.Identity,
                                                     bias=nm4b[:, c:c + 1],
                                                     scale=rs4b[:, c:c + 1])
                                nc.scalar.activation(c3e[:], msgt[:, c, :],
                                                     AF.Identity,
                                                     bias=nm3b[:, c:c + 1],
                                                     scale=rs3b[:, c:c + 1])
                            else:
                                nc.vector.tensor_scalar(
                                    out=c2e[:], in0=c2pb[:, c, :],
                                    scalar1=mv4b[:, 2 * c:2 * c + 1],
                                    scalar2=rs4b[:, c:c + 1],
                                    op0=OP.subtract, op1=OP.mult)
                                nc.vector.tensor_mul(c2e[:], c2e[:], g22[:])
                                nc.vector.tensor_add(c2e[:], c2e[:], be22[:])
                                nc.vector.tensor_scalar(
                                    out=c3e[:], in0=msgt[:, c, :],
                                    scalar1=mv3b[:, 2 * c:2 * c + 1],
                                    scalar2=rs3b[:, c:c + 1],
                                    op0=OP.subtract, op1=OP.mult)
                                nc.vector.tensor_mul(c3e[:], c3e[:], g32[:])
                                nc.vector.tensor_add(c3e[:], c3e[:], be32[:])
                            acc = bsm.tile([P, P], f32, tag="acc")
                            nc.vector.tensor_add(acc[:], c2e[:], c3e[:])
                            nc.vector.tensor_add(acc[:], acc[:], edt[:, c, :])
                            nc.scalar.activation(outt[:, c, :], acc[:], AF.Tanh)
                        nc.sync.dma_start(
                            out=out[q * E_CAP + e0:q * E_CAP + e0 + ne, :]
                            .rearrange("(n p) f -> p n f", p=P), in_=outt[:])
                        c0 += nch
    nc.finalize()
    return nc


def kernel(**inputs):
    from concourse.bass_utils import run_bass_kernel_spmd

    i = np.asarray(inputs["i"]).astype(np.int64)
    j = np.asarray(inputs["j"]).astype(np.int64)
    idx_i = np.asarray(inputs["index_i"]).astype(np.int64)
    idx_j = np.asarray(inputs["index_j"]).astype(np.int64)
    idx_k = np.asarray(inputs["index_k"]).astype(np.int64)
    ji = np.asarray(inputs["index_ji"]).astype(np.int64)
    kj = np.asarray(inputs["index_kj"]).astype(np.int64)
    node = np.asarray(inputs["node_embedding"], np.float32)
    edge = np.asarray(inputs["edge_embedding"], np.float32)

    node_h = node.astype(np.float16)
    edge_ha = edge.astype(np.float16)
    w3f = np.vstack([np.asarray(inputs["w_c3"], np.float32),
                     np.asarray(inputs["b_c3"], np.float32)[None]])
    w2f = np.vstack([np.asarray(inputs["w_c2"], np.float32),
                     np.asarray(inputs["b_c2"], np.float32)[None]])
    w3x = w3f.astype(np.float16)
    w2x = w2f.astype(np.float16)
    gbe3 = np.stack([np.asarray(inputs["g_bn_c2"], np.float32),
                     np.asarray(inputs["be_bn_c2"], np.float32),
                     np.asarray(inputs["g_bn_c3"], np.float32),
                     np.asarray(inputs["be_bn_c3"], np.float32)])
    gbe2 = np.stack([np.asarray(inputs["g_bn_c2_2"], np.float32),
                     np.asarray(inputs["be_bn_c2_2"], np.float32),
                     np.asarray(inputs["g_bn_c3_2"], np.float32),
                     np.asarray(inputs["be_bn_c3_2"], np.float32)])
    fast = (np.all(gbe3[0] == 1) and np.all(gbe3[2] == 1)
            and np.all(gbe2[0] == 1) and np.all(gbe2[2] == 1)
            and np.all(gbe3[1] == 0) and np.all(gbe3[3] == 0)
            and np.all(gbe2[1] == 0) and np.all(gbe2[3] == 0))

    order = np.argsort(ji, kind="stable")
    ji_sorted = ji[order]

    in_maps = []
    for m in range(N_CORES):
        d = _prep_core(m, i, j, idx_i, idx_j, idx_k, ji, kj, order, ji_sorted)
        egrid = np.zeros((Q * E_CAP, DE), np.float16)
        for q in range(Q):
            base = m * E_SH + E_OFF[q]
            egrid[q * E_CAP:q * E_CAP + E_THIRD[q]] = \
                edge_ha[base:base + E_THIRD[q]]
        d.update(iot=np.tile(np.arange(P, dtype=np.float16), (P, 1)),
                 node_h=node_h, edge_h=edge_ha,
                 edge_sh=edge_ha[m * E_SH:(m + 1) * E_SH].copy(),
                 edge_f=egrid, w3=w3x, w2=w2x, gbe3=gbe3, gbe2=gbe2)
        in_maps.append(d)

    key = ("k", fast)
    if key not in _CACHE:
        _CACHE[key] = _build_kernel(fast)
    nc = _CACHE[key]

    import os
    trace = bool(os.environ.get("KERNEL_TRACE"))
    res = run_bass_kernel_spmd(nc, in_maps, core_ids=list(range(N_CORES)),
                               trace=trace)
    global LAST_RESULT
    LAST_RESULT = res

    full = np.zeros((N_EDGES, DE), np.float32)
    for m in range(N_CORES):
        o = res.results[m]["out"].astype(np.float32)
        for q in range(Q):
            base = m * E_SH + E_OFF[q]
            full[base:base + E_THIRD[q]] = o[q * E_CAP:q * E_CAP + E_THIRD[q]]
    return full
